# revision 14
# baseline (speedup 1.0000x reference)
"""Multi-head attention (B=4, S=2048, D=1024, H=16, HD=64) on 8 trn2 cores.

Sharding: core c -> (batch b = c//2, query-half = c%2). Each core computes
K/V for the full sequence of its batch (duplicated across the pair; avoids
collectives) and attention + output projection for its 1024 queries.

Device pipeline per core (bf16 matmul operands, fp32 PSUM accumulation):
  1. x [2048,1024] -> xT chunks [128d, 2048t] via PE transposes.
  2. Per head-group g (4 heads, 256 e-cols):
       KT_g [2x128e, 2048t], QT_g [2x128e, 1024t] (transposed projections,
       W chunk stationary), V_g [128t, 16kc, 4*65] (natural projection,
       xT chunk stationary; col 64 of each 65-block = 1.0 for the softmax
       denominator). Biases added via rank-1 PE accumulate.
  3. Attention per (head, 512-query group): scoresT [128k, 512q] chunks on
     PE -> exp on ACT (scale=1/8, no max subtraction: scores ~ N(0,1)) ->
     PV accumulate outT [65, 512] (V_aug stationary). Row 64 = sum(exp).
     Normalize: reciprocal (DVE) -> rank-1 broadcast (PE) -> multiply into
     attn_outT [din, q] (DVE).
  4. Output projection: attn_outT chunk stationary, W_out moving ->
     out [128q, 512] + rank-1 bias -> DMA.
"""

import numpy as np
import sys

for p in ("/opt/trn_rl_repo", "/opt/pypackages"):
    if p not in sys.path:
        sys.path.append(p)

import concourse.bass as bass
import concourse.bacc as bacc
import concourse.mybir as mybir
from concourse import tile
from concourse.bass_utils import run_bass_kernel_spmd

F32 = mybir.dt.float32
F32R = mybir.dt.float32r
BF16 = mybir.dt.bfloat16
EXP = mybir.ActivationFunctionType.Exp

B, S, D, H, HD = 4, 2048, 1024, 16, 64
SQ = 1024          # queries per core
NT = S // 128      # 16 token tiles (full seq)
NTQ = SQ // 128    # 8 query token tiles
ND = D // 128      # 8 contraction chunks
GH = 4             # heads per group
NG = H // GH       # 4 groups
ES = GH * HD       # 256 e-cols per group
VW = GH * 65       # 260: V block width incl. ones columns

N_CORES = 8


def _build():
    nc = bacc.Bacc(
        "TRN2",
        target_bir_lowering=False,
        debug=False,
        enable_asserts=True,
        num_devices=N_CORES,
    )
    x_d = nc.dram_tensor("x", [S, D], BF16, kind="ExternalInput")
    wqkv_d = nc.dram_tensor("w_qkv", [D, 3 * D], BF16, kind="ExternalInput")
    bqkv_d = nc.dram_tensor("b_qkv", [1, 3 * D], BF16, kind="ExternalInput")
    wout_d = nc.dram_tensor("w_out", [D, D], BF16, kind="ExternalInput")
    bout_d = nc.dram_tensor("b_out", [1, D], BF16, kind="ExternalInput")
    out_d = nc.dram_tensor("out", [SQ, D], F32, kind="ExternalOutput")

    with tile.TileContext(nc, trace_sim=False) as tc:
        with (
            tc.tile_pool(name="const", bufs=1) as constp,
            tc.tile_pool(name="xt", bufs=1) as xtp,
            tc.tile_pool(name="att", bufs=1) as attp,
            tc.tile_pool(name="kt", bufs=2) as ktp,
            tc.tile_pool(name="qt", bufs=2) as qtp,
            tc.tile_pool(name="vg", bufs=2) as vgp,
            tc.tile_pool(name="wt", bufs=3) as wtp,
            tc.tile_pool(name="etx", bufs=4) as etxp,
            tc.tile_pool(name="rsb", bufs=2) as rsbp,
            tc.tile_pool(name="bqg", bufs=2) as bqgp,
            tc.tile_pool(name="ps_sc", bufs=2, space="PSUM") as ps_sc,
            tc.tile_pool(name="ps_ot", bufs=2, space="PSUM") as ps_ot,
            tc.tile_pool(name="ps_pj", bufs=2, space="PSUM") as ps_pj,
        ):
            onesb = constp.tile([128, 512], F32, name="onesb")
            nc.vector.memset(onesb[:], 1.0)
            ones = constp.tile([1, 512], BF16, name="ones")
            nc.vector.tensor_copy(ones[:], onesb[0:1, :])
            onesr = constp.tile([1, 64], F32R, name="onesr")
            nc.vector.tensor_copy(onesr[:], onesb[0:1, 0:64])
            bo_sb = constp.tile([1, D], BF16, name="bo_sb")
            nc.sync.dma_start(bo_sb[:], bout_d[:])
            bob = constp.tile([128, D], F32, name="bob")
            for dg in range(2):
                pjb = ps_pj.tile([128, 512], F32, name="pjb", tag="pj")
                nc.tensor.matmul(
                    pjb[:], ones[0:1, 0:128],
                    bo_sb[0:1, dg * 512:(dg + 1) * 512],
                    start=True, stop=True,
                )
                nc.vector.tensor_copy(bob[:, dg * 512:(dg + 1) * 512], pjb[:])

            xts = [xtp.tile([128, S], BF16, name=f"xt{c}") for c in range(ND)]
            atts = [attp.tile([128, SQ], BF16, name=f"at{c}") for c in range(ND)]

            # ---- phase 1: DMA-transpose x into d-major chunks --------------
            for tg in range(4):
                for dc in range(ND):
                    nc.sync.dma_start_transpose(
                        out=xts[dc][:, tg * 512:(tg + 1) * 512],
                        in_=x_d[tg * 512:(tg + 1) * 512,
                                dc * 128:(dc + 1) * 128],
                    )

            # ---- phase 2: per-group projections + attention ----------------
            for g in range(NG):
                qoff = g * ES
                koff = D + g * ES
                voff = 2 * D + g * ES

                bqcb = bqgp.tile([128, 4], BF16, name="bqcb", tag="bqcb")
                for et in range(2):
                    nc.gpsimd.dma_start(
                        bqcb[:, et:et + 1],
                        bqkv_d[0:1, koff + et * 128:koff + (et + 1) * 128],
                    )
                    nc.gpsimd.dma_start(
                        bqcb[:, 2 + et:3 + et],
                        bqkv_d[0:1, qoff + et * 128:qoff + (et + 1) * 128],
                    )
                bqc = bqgp.tile([128, 4], F32, name="bqc", tag="bqc")
                nc.vector.tensor_copy(bqc[:], bqcb[:])

                # K projection (transposed): KT [e, t]
                wk = []
                for dc in range(ND):
                    w = wtp.tile([128, ES], BF16, name=f"wk{dc}", tag=f"w{dc}")
                    nc.gpsimd.dma_start(
                        w[:], wqkv_d[dc * 128:(dc + 1) * 128, koff:koff + ES]
                    )
                    wk.append(w)
                kts = [
                    ktp.tile([128, S], BF16, name=f"kt{e}", tag=f"kt{e}")
                    for e in range(2)
                ]
                for tg in range(4):
                    for et in range(2):
                        pj = ps_pj.tile([128, 512], F32, name="pjk", tag="pj")
                        for dc in range(ND):
                            nc.tensor.matmul(
                                pj[:],
                                wk[dc][:, et * 128:(et + 1) * 128],
                                xts[dc][:, tg * 512:(tg + 1) * 512],
                                start=(dc == 0),
                                stop=(dc == ND - 1),
                            )
                        nc.vector.tensor_scalar_add(
                            kts[et][:, tg * 512:(tg + 1) * 512], pj[:],
                            bqc[:, et:et + 1],
                        )

                # Q projection (transposed, local 1024 tokens only)
                wq = []
                for dc in range(ND):
                    w = wtp.tile([128, ES], BF16, name=f"wq{dc}", tag=f"w{dc}")
                    nc.gpsimd.dma_start(
                        w[:], wqkv_d[dc * 128:(dc + 1) * 128, qoff:qoff + ES]
                    )
                    wq.append(w)
                qts = [
                    qtp.tile([128, SQ], BF16, name=f"qt{e}", tag=f"qt{e}")
                    for e in range(2)
                ]
                for tg in range(2):
                    for et in range(2):
                        pj = ps_pj.tile([128, 512], F32, name="pjq", tag="pj")
                        for dc in range(ND):
                            nc.tensor.matmul(
                                pj[:],
                                wq[dc][:, et * 128:(et + 1) * 128],
                                xts[dc][:, tg * 512:(tg + 1) * 512],
                                start=(dc == 0),
                                stop=(dc == ND - 1),
                            )
                        nc.vector.tensor_scalar_add(
                            qts[et][:, tg * 512:(tg + 1) * 512], pj[:],
                            bqc[:, 2 + et:3 + et],
                        )

                # V projection (natural): two groups at once, N=512
                if g % 2 == 0:
                    wv = []
                    for dc in range(ND):
                        w = wtp.tile(
                            [128, 2 * ES], BF16, name=f"wv{dc}", tag=f"wv{dc}"
                        )
                        nc.gpsimd.dma_start(
                            w[:],
                            wqkv_d[dc * 128:(dc + 1) * 128, voff:voff + 2 * ES],
                        )
                        wv.append(w)
                    bqg2 = bqgp.tile([1, 2 * ES], BF16, name="bqg2", tag="bqg2")
                    nc.gpsimd.dma_start(
                        bqg2[0:1, :], bqkv_d[0:1, voff:voff + 2 * ES]
                    )
                    bvb = rsbp.tile([128, 2 * ES], F32, name="bvb", tag="bvb")
                    pjv = ps_pj.tile([128, 512], F32, name="pjvb", tag="pj")
                    nc.tensor.matmul(
                        pjv[:], ones[0:1, 0:128], bqg2[0:1, :],
                        start=True, stop=True,
                    )
                    nc.vector.tensor_copy(bvb[:], pjv[:])
                    vg = vgp.tile([128, NT, 2 * VW], BF16, name="vg", tag="vg")
                    nc.vector.tensor_copy(
                        vg.rearrange("p t (h x) -> p t h x", h=2 * GH)[:, :, :, 64:65],
                        onesb[:, 0:128].rearrange(
                            "p (t h x) -> p t h x", t=NT, h=2 * GH
                        ),
                    )
                    for ti in range(NT):
                        pj = ps_pj.tile([128, 512], F32, name="pjv", tag="pj")
                        for dc in range(ND):
                            nc.tensor.matmul(
                                pj[:],
                                xts[dc][:, ti * 128:(ti + 1) * 128],
                                wv[dc][:],
                                start=(dc == 0),
                                stop=(dc == ND - 1),
                            )
                        nc.vector.tensor_add(
                            vg[:, ti].rearrange("p (h x) -> p h x", h=2 * GH)[:, :, 0:64],
                            pj.rearrange("p (h x) -> p h x", h=2 * GH),
                            bvb.rearrange("p (h x) -> p h x", h=2 * GH),
                        )

                # Attention for the group's 4 heads
                for hh in range(GH):
                    h = g * GH + hh
                    kt_t = kts[hh // 2]
                    qt_t = qts[hh // 2]
                    po = (hh % 2) * 64
                    at_t = atts[h // 2]
                    apo = (h % 2) * 64
                    for qg in range(2):
                        ot = ps_ot.tile([65, 512], F32, name="ot", tag="ot")
                        for kc2 in range(8):
                            sc = ps_sc.tile(
                                [128, 1024], F32, name="sc", tag="sc"
                            )
                            for j in range(2):
                                kc = kc2 * 2 + j
                                nc.tensor.matmul(
                                    sc[:, j * 512:(j + 1) * 512],
                                    kt_t[po:po + 64, kc * 128:(kc + 1) * 128],
                                    qt_t[po:po + 64, qg * 512:(qg + 1) * 512],
                                    start=True,
                                    stop=True,
                                )
                            et_t = etxp.tile([128, 1024], BF16, name="et", tag="et")
                            nc.scalar.activation(
                                et_t[:], sc[:], EXP, scale=0.125
                            )
                            for j in range(2):
                                kc = kc2 * 2 + j
                                hv = hh + GH * (g % 2)
                                nc.tensor.matmul(
                                    ot[:],
                                    vg[:, kc, hv * 65:(hv + 1) * 65],
                                    et_t[:, j * 512:(j + 1) * 512],
                                    start=(kc == 0),
                                    stop=(kc == 15),
                                )
                        rsb = rsbp.tile([1, 512], F32R, name="rsb", tag="rsb")
                        with nc.allow_low_precision(reason="f32r recip"):
                            nc.vector.reciprocal(rsb[:], ot[64:65, :])
                        rc = ps_pj.tile([64, 512], F32, name="rc", tag="pj")
                        nc.tensor.matmul(
                            rc[:],
                            onesr[:],
                            rsb[:],
                            start=True,
                            stop=True,
                        )
                        rcs = rsbp.tile([64, 512], F32, name="rcs", tag="rcs")
                        nc.vector.tensor_copy(rcs[:], rc[:])
                        nc.vector.tensor_mul(
                            at_t[apo:apo + 64, qg * 512:(qg + 1) * 512],
                            ot[0:64, :],
                            rcs[:],
                        )

            # ---- phase 3: output projection --------------------------------
            wo = []
            for dc in range(ND):
                w = xtp.tile([128, D], BF16, name=f"wo{dc}", tag=f"xt{dc}")
                nc.sync.dma_start(w[:], wout_d[dc * 128:(dc + 1) * 128, :])
                wo.append(w)
            for qt_i in range(NTQ):
                for dg in range(2):
                    pj = ps_pj.tile([128, 512], F32, name="pjo", tag="pj")
                    for dc in range(ND):
                        nc.tensor.matmul(
                            pj[:],
                            atts[dc][:, qt_i * 128:(qt_i + 1) * 128],
                            wo[dc][:, dg * 512:(dg + 1) * 512],
                            start=(dc == 0),
                            stop=(dc == ND - 1),
                        )
                    ob = etxp.tile([128, 512], F32, name="ob", tag="et")
                    nc.vector.tensor_add(ob[:], pj[:], bob[:, dg * 512:(dg + 1) * 512])
                    nc.sync.dma_start(
                        out_d[qt_i * 128:(qt_i + 1) * 128,
                              dg * 512:(dg + 1) * 512],
                        ob[:],
                    )

    nc.compile()
    return nc


_NC = None


def make_in_maps(x, W_qkv, b_qkv, W_out, b_out):
    import ml_dtypes

    bf16 = ml_dtypes.bfloat16
    x = np.asarray(x, dtype=np.float32).astype(bf16)
    W_qkv = np.ascontiguousarray(np.asarray(W_qkv, np.float32).astype(bf16))
    b_qkv = np.ascontiguousarray(
        np.asarray(b_qkv, np.float32).reshape(1, 3 * D).astype(bf16)
    )
    W_out = np.ascontiguousarray(np.asarray(W_out, np.float32).astype(bf16))
    b_out = np.ascontiguousarray(
        np.asarray(b_out, np.float32).reshape(1, D).astype(bf16)
    )

    in_maps = []
    for c in range(N_CORES):
        b_i, half = c // 2, c % 2
        xb = x[b_i]
        # local query-half first, other half after (k-order is irrelevant)
        x_r = np.ascontiguousarray(
            np.concatenate(
                [xb[half * SQ:(half + 1) * SQ], xb[(1 - half) * SQ:(2 - half) * SQ]],
                axis=0,
            )
        )
        in_maps.append(
            {
                "x": x_r,
                "w_qkv": W_qkv,
                "b_qkv": b_qkv,
                "w_out": W_out,
                "b_out": b_out,
            }
        )
    return in_maps


def kernel(x, W_qkv, b_qkv, W_out, b_out):
    global _NC
    if _NC is None:
        _NC = _build()
    in_maps = make_in_maps(x, W_qkv, b_qkv, W_out, b_out)

    res = run_bass_kernel_spmd(_NC, in_maps, list(range(N_CORES))).results
    out = np.empty((B, S, D), dtype=np.float32)
    for c in range(N_CORES):
        b_i, half = c // 2, c % 2
        out[b_i, half * SQ:(half + 1) * SQ] = res[c]["out"]
    return out
